# revision 27
# baseline (speedup 1.0000x reference)
"""CapsuleLayer (dynamic routing, 3 iterations) Trainium2 Bass kernel.

Problem (hardcoded):
    x: [64, 2048, 8] f32, W: [2048, 32, 8, 16] f32
    u_hat[b,o,i,k] = sum_d x[b,i,d] * W[i,o,d,k]
    3 rounds of routing-by-agreement over logits b[B,O,I], softmax over O.
    out v: [64, 32, 16] f32.

Sharding: data-parallel over batch across 8 cores (8 batch rows each), W
replicated. Per-core layout: partitions = (g,b) with g=16 i's per tile,
free = (k,o); T=128 tiles of 16 i's.

Pass 0: per tile, PE matmul with block-diag x produces u_hat tile; a second
accumulating matmul (lhsT = x/32) produces s0 directly. PSUM->SBUF bf16
copies split between ACT and DVE. W DMA'd on the Sync queue 4 tiles/DMA.

Rounds 1,2 (software-pipelined, batch of 8 tiles, skew-1 tail):
  V: vu = u*v (bcast), k-tree reduce -> logits; S: batched exp (no accum);
  V-tail: z = reduce_sum(e), rz = 1/z, c = e*rz (bcast), cu = u*c;
  T-tail: 8 ones-matmuls accumulate s in PSUM.
Squash uses ln/exp instead of sqrt (one ACT table set for the whole kernel).
"""

import numpy as np
import ml_dtypes

BF16 = ml_dtypes.bfloat16

B, I, D, O, K = 64, 2048, 8, 32, 16
NC_N = 8              # cores
BL = B // NC_N        # 8 batch rows per core
G = 16                # i's per tile
T = I // G            # 128 tiles
FREE = O * K          # 512, layout (k,o): col = k*32+o
EPS = 1e-7
B8 = 16               # tiles per round-loop iteration
NB = T // B8          # iterations per round
WQ = 4                # W tiles per DMA

_CACHE = {}


def _patch_act_tables():
    """Constrain Exp/Ln/Square/Identity to the one ACT table set that has
    them all, so squash (Ln,Exp) and the rounds (Exp) never swap table
    sets. Set positions are preserved so emitted set ids stay valid."""
    import functools
    import concourse.hw_specs as hw
    import concourse.mybir as mybir
    if getattr(hw, "_capsule_act_patch", False):
        return
    A = mybir.ActivationFunctionType
    orig = hw.get_activation_tables

    @functools.cache
    def patched(arch):
        keep = {A.Exp, A.Ln, A.Square, A.Identity}
        out = {}
        for name, fns in orig(arch).items():
            if name == "natural_log_exp_and_others":
                out[name] = set(fns)
            else:
                out[name] = set(fns) - keep
        return out

    hw.get_activation_tables = patched
    # bacc binds the name at import time; patch that reference too
    import concourse.bacc as bacc
    bacc.get_activation_tables = patched
    hw._capsule_act_patch = True


def _build_bass():
    import concourse.bass as bass
    import concourse.bacc as bacc
    import concourse.mybir as mybir
    import concourse.tile as tile

    _patch_act_tables()

    f32 = mybir.dt.float32
    bf16 = mybir.dt.bfloat16
    nc = bacc.Bacc()

    wd = nc.dram_tensor("w", [T // WQ, 128, WQ * FREE], bf16, kind="ExternalInput")
    xtd = nc.dram_tensor("xt", [128, T, BL], bf16, kind="ExternalInput")
    xblkd = nc.dram_tensor("xblk", [128, T, 128], bf16, kind="ExternalInput")
    onesd = nc.dram_tensor("ones", [128, BL], bf16, kind="ExternalInput")
    onestd = nc.dram_tensor("onest", [BL, 128], bf16, kind="ExternalInput")
    outd = nc.dram_tensor("out", [BL, FREE], f32, kind="ExternalOutput")

    AX = mybir.AxisListType
    ALU = mybir.AluOpType
    ACTF = mybir.ActivationFunctionType

    with tile.TileContext(nc) as tc:
        with (
            tc.tile_pool(name="const", bufs=1) as constp,
            tc.tile_pool(name="u16", bufs=1) as up,
            tc.tile_pool(name="vexp", bufs=1) as vexpp,
            tc.tile_pool(name="psum_s", bufs=2, space="PSUM") as psum_s,
            tc.tile_pool(name="psum_v", bufs=1, space="PSUM") as psum_v,
        ):
            eps_sb = constp.tile([128, 1], f32)
            nc.gpsimd.memset(eps_sb[:], EPS)
            xt_sb = constp.tile([128, T, BL], bf16)
            ones_sb = constp.tile([128, BL], bf16)
            onest_sb = constp.tile([BL, 128], bf16)

            u16 = up.tile([128, T, FREE], bf16)

            # ---------------- pass 0: u_hat + s0 ----------------
            s0_ps = psum_s.tile([BL, FREE], f32, tag="s_ps")
            with (
                tc.tile_pool(name="xblk", bufs=1) as xblkp,
                tc.tile_pool(name="wt", bufs=4) as wtp,
                tc.tile_pool(name="psum_u", bufs=4, space="PSUM") as psum_u,
            ):
                # block-diag x built host-side: xblk[g*8+d, t, g*8+b] = x[b, t*16+g, d]
                # chunked so tile-0's stationary doesn't wait on a 4 MiB DMA
                # first chunk small so tile-0's stationary arrives fast
                xch = [8, 40, 40, 40]
                xoff = [0, 8, 48, 88]
                xblk_c = []
                for q in range(4):
                    xc = xblkp.tile([128, xch[q], 128], bf16, tag=f"xblk{q}")
                    # gpsimd (SWDGE) queue: runs parallel to the W DMAs on sync
                    nc.gpsimd.dma_start(
                        xc[:], xblkd[:, xoff[q]:xoff[q] + xch[q], :])
                    xblk_c.append(xc)

                def xblk_ap(t):
                    q = 0 if t < 8 else 1 + (t - 8) // 40
                    return xblk_c[q][:, t - xoff[q], :]
                wt4 = None
                for t in range(T):
                    q, j = divmod(t, WQ)
                    if j == 0:
                        wt4 = wtp.tile([128, WQ, FREE], bf16, tag="wt4")
                        nc.sync.dma_start(wt4[:], wd[q])
                        if q == 0:
                            # consts ride the sync queue behind W chunk 0 so
                            # the first matmul isn't gated on them
                            nc.sync.dma_start(xt_sb[:], xtd[:])
                            nc.sync.dma_start(ones_sb[:], onesd[:])
                            nc.sync.dma_start(onest_sb[:], onestd[:])
                    ut_ps = psum_u.tile([128, FREE], f32, tag="u_ps")
                    nc.tensor.matmul(ut_ps[:], xblk_ap(t), wt4[:, j, :])
                    # s0 accumulation straight from x/32, W (fp32-exact in PSUM)
                    nc.tensor.matmul(
                        s0_ps[:], xt_sb[:, t, :], wt4[:, j, :],
                        start=(t == 0), stop=(t == T - 1),
                    )
                    # PSUM -> SBUF bf16 cast copy, weighted split ACT/DVE
                    if t % 7 < 4:
                        nc.scalar.copy(u16[:, t, :], ut_ps[:])
                    else:
                        nc.vector.tensor_copy(u16[:, t, :], ut_ps[:])

            # ---------------- squash (ln/exp; no sqrt table) ----------------
            with tc.tile_pool(name="sq", bufs=1) as sqp:

                def squash_and_bcast(s_ps, last, tag="v0"):
                    """v = squash(s_ps); write vexp [128, FREE] bf16, or DMA
                    fp32 v to outd if last. Returns vexp tile or None."""
                    sq2 = sqp.tile([BL, O, K], f32, tag="sq2")
                    nc.scalar.activation(
                        sq2[:], s_ps[:].rearrange("p (k o) -> p o k", o=O),
                        ACTF.Square)
                    s2 = sqp.tile([BL, O], f32, tag="s2")
                    nc.vector.reduce_sum(s2[:], sq2[:], axis=AX.X)
                    lnt = sqp.tile([BL, O], f32, tag="lnt")
                    nc.scalar.activation(lnt[:], s2[:], ACTF.Ln, bias=eps_sb[:BL])
                    rt = sqp.tile([BL, O], f32, tag="rt")
                    nc.scalar.activation(rt[:], lnt[:], ACTF.Exp, scale=0.5)
                    # den = (s2 + 1) * rt
                    den = sqp.tile([BL, O], f32, tag="den")
                    nc.vector.scalar_tensor_tensor(
                        den[:], s2[:], 1.0, rt[:], ALU.add, ALU.mult)
                    rden = sqp.tile([BL, O], f32, tag="rden")
                    nc.vector.reciprocal(rden[:], den[:])
                    scl = sqp.tile([BL, O], f32, tag="scl")
                    nc.vector.tensor_mul(scl[:], s2[:], rden[:])
                    # v = s * scl (broadcast over k)
                    v = sqp.tile([BL, K, O], f32 if last else bf16, tag="v")
                    nc.vector.tensor_mul(
                        v[:], s_ps[:].rearrange("p (k o) -> p k o", o=O),
                        scl[:].unsqueeze(1).broadcast_to([BL, K, O]))
                    if last:
                        nc.sync.dma_start(outd[:], v[:].rearrange("p k o -> p (k o)"))
                        return None
                    # replicate v to all 16 partition groups via PE
                    vrep_ps = psum_v.tile([128, FREE], f32, tag="vrep")
                    nc.tensor.matmul(
                        vrep_ps[:], onest_sb[:],
                        v[:].rearrange("p k o -> p (k o)"))
                    vexp1 = vexpp.tile([128, FREE], bf16, tag=tag)
                    nc.scalar.copy(vexp1[:], vrep_ps[:])
                    return vexp1

                vexp0 = squash_and_bcast(s0_ps, last=False, tag="v0")

                # ---------------- rounds 1, 2 (pipelined) ----------------
                with (
                    tc.tile_pool(name="scr", bufs=1) as scr,
                    tc.tile_pool(name="epool", bufs=2) as ep,
                    tc.tile_pool(name="cupool", bufs=2) as cup,
                ):
                    for rnd in (1, 2):
                        if rnd == 1:
                            vin = vexp0
                        else:
                            # b2 = u.(v0+v1): one vu pass against v0+v1
                            vin = vexpp.tile([128, FREE], bf16, tag="vsum")
                            nc.vector.tensor_add(vin[:], vexp0[:], vexp1[:])
                        s_ps = psum_s.tile([BL, FREE], f32, tag="s_ps")
                        vview = vin[:].rearrange(
                            "p (k o) -> p k o", o=O).unsqueeze(1).broadcast_to(
                            [128, B8, K, O])
                        pending = None

                        CH = B8 // 2

                        def tail(p):
                            tb, e_t = p
                            z = scr.tile([128, B8], f32, tag="z")
                            nc.vector.reduce_sum(z[:], e_t[:], axis=AX.X)
                            rz = scr.tile([128, B8], f32, tag="rz")
                            nc.vector.reciprocal(rz[:], z[:])
                            c = scr.tile([128, B8, O], bf16, tag="c")
                            nc.vector.tensor_mul(
                                c[:], e_t[:],
                                rz[:].unsqueeze(-1).broadcast_to([128, B8, O]))
                            # two half-chunks through a 2-deep rotating pool:
                            # matmuls of chunk N never block chunk N+1's write
                            for ch in range(2):
                                j0 = ch * CH
                                cu = cup.tile([128, CH, K, O], bf16, tag="cu")
                                nc.vector.tensor_mul(
                                    cu[:],
                                    u16[:, tb + j0:tb + j0 + CH, :].rearrange(
                                        "p t (k o) -> p t k o", o=O),
                                    c[:, j0:j0 + CH].unsqueeze(2).broadcast_to(
                                        [128, CH, K, O]))
                                for jj in range(CH):
                                    t = tb + j0 + jj
                                    nc.tensor.matmul(
                                        s_ps[:], ones_sb[:],
                                        cu[:, jj, :, :].rearrange(
                                            "p k o -> p (k o)"),
                                        start=(t == 0), stop=(t == T - 1))

                        for ti in range(NB):
                            tb = ti * B8
                            u_sl = u16[:, tb:tb + B8, :].rearrange(
                                "p t (k o) -> p t k o", o=O)
                            vu = scr.tile([128, B8, K, O], bf16, tag="vu")
                            nc.vector.tensor_mul(vu[:], u_sl, vview)
                            t1 = scr.tile([128, B8, 8, O], bf16, tag="t1")
                            nc.vector.tensor_add(
                                t1[:], vu[:, :, 0:8], vu[:, :, 8:16])
                            t2 = scr.tile([128, B8, 4, O], bf16, tag="t2")
                            nc.vector.tensor_add(
                                t2[:], t1[:, :, 0:4], t1[:, :, 4:8])
                            t3 = scr.tile([128, B8, 2, O], bf16, tag="t3")
                            nc.vector.tensor_add(
                                t3[:], t2[:, :, 0:2], t2[:, :, 2:4])
                            lg = scr.tile([128, B8, O], bf16, tag="lg")
                            nc.vector.tensor_add(
                                lg[:], t3[:, :, 0, :], t3[:, :, 1, :])
                            e_t = ep.tile([128, B8, O], bf16, tag="e")
                            nc.scalar.activation(e_t[:], lg[:], ACTF.Exp)
                            if pending is not None:
                                tail(pending)
                            pending = (tb, e_t)
                        tail(pending)
                        vexp1 = squash_and_bcast(
                            s_ps, last=(rnd == 2), tag="v1")
    nc.finalize()
    return nc


def _host_prep():
    """Core-independent input prep pieces."""
    ones = np.zeros((128, BL), dtype=BF16)
    for g in range(G):
        for b in range(BL):
            ones[g * 8 + b, b] = 1
    onest = np.ascontiguousarray(ones.T)
    return ones, onest


def kernel(x: np.ndarray, W: np.ndarray) -> np.ndarray:
    from concourse import bass_utils

    if "nc" not in _CACHE:
        _CACHE["nc"] = _build_bass()
        _CACHE["ones"], _CACHE["onest"] = _host_prep()
    nc = _CACHE["nc"]

    # W -> [T, (g,d), (k,o)] : w[t, g*8+d, k*32+o] = W[t*16+g, o, d, k]
    # then group 4 tiles per DMA: [T//4, 128, 4*FREE]
    wr = (W.reshape(T, G, O, D, K).transpose(0, 1, 3, 4, 2)
          .reshape(T, 128, FREE))
    wr4 = np.ascontiguousarray(
        wr.reshape(T // WQ, WQ, 128, FREE).transpose(0, 2, 1, 3)
        .reshape(T // WQ, 128, WQ * FREE)).astype(BF16)
    in_maps = []
    for c in range(NC_N):
        xl = x[c * BL:(c + 1) * BL]  # [8, 2048, 8]
        # xt[g*8+d, t, b] = xl[b, t*16+g, d] / 32  (folds round-0 c=1/O)
        xt = np.ascontiguousarray(
            xl.reshape(BL, T, G, D).transpose(2, 3, 1, 0).reshape(128, T, BL)
        ) * (1.0 / O)
        xt = xt.astype(BF16)
        xtu = np.ascontiguousarray(
            xl.reshape(BL, T, G, D).transpose(2, 3, 1, 0).reshape(128, T, BL)
        ).astype(BF16)
        xblk = np.zeros((128, T, 128), dtype=BF16)
        for g in range(G):
            xblk[g * 8:(g + 1) * 8, :, g * 8:(g + 1) * 8] = xtu[g * 8:(g + 1) * 8]
        in_maps.append({"w": wr4, "xt": xt, "xblk": xblk, "ones": _CACHE["ones"],
                        "onest": _CACHE["onest"]})

    _CACHE["in_maps"] = in_maps
    res = bass_utils.run_bass_kernel_spmd(nc, in_maps, core_ids=list(range(NC_N)))
    out = np.empty((B, O, K), np.float32)
    for c in range(NC_N):
        v = res.results[c]["out"].reshape(BL, K, O)  # (k,o) cols
        out[c * BL:(c + 1) * BL] = v.transpose(0, 2, 1)
    return out


# revision 29
# speedup vs baseline: 1.0061x; 1.0061x over previous
"""CapsuleLayer (dynamic routing, 3 iterations) Trainium2 Bass kernel.

Problem (hardcoded):
    x: [64, 2048, 8] f32, W: [2048, 32, 8, 16] f32
    u_hat[b,o,i,k] = sum_d x[b,i,d] * W[i,o,d,k]
    3 rounds of routing-by-agreement over logits b[B,O,I], softmax over O.
    out v: [64, 32, 16] f32.

Sharding: data-parallel over batch across 8 cores (8 batch rows each), W
replicated. Per-core layout: partitions = (g,b) with g=16 i's per tile,
free = (k,o); T=128 tiles of 16 i's.

Pass 0: per tile, PE matmul with block-diag x produces u_hat tile; a second
accumulating matmul (lhsT = x/32) produces s0 directly. PSUM->SBUF bf16
copies split between ACT and DVE. W DMA'd on the Sync queue 4 tiles/DMA.

Rounds 1,2 (software-pipelined, batch of 8 tiles, skew-1 tail):
  V: vu = u*v (bcast), k-tree reduce -> logits; S: batched exp (no accum);
  V-tail: z = reduce_sum(e), rz = 1/z, c = e*rz (bcast), cu = u*c;
  T-tail: 8 ones-matmuls accumulate s in PSUM.
Squash uses ln/exp instead of sqrt (one ACT table set for the whole kernel).
"""

import numpy as np
import ml_dtypes

BF16 = ml_dtypes.bfloat16

B, I, D, O, K = 64, 2048, 8, 32, 16
NC_N = 8              # cores
BL = B // NC_N        # 8 batch rows per core
G = 16                # i's per tile
T = I // G            # 128 tiles
FREE = O * K          # 512, layout (k,o): col = k*32+o
EPS = 1e-7
B8 = 16               # tiles per round-loop iteration
NB = T // B8          # iterations per round
WQ = 4                # W tiles per DMA

_CACHE = {}


def _patch_act_tables():
    """Constrain Exp/Ln/Square/Identity to the one ACT table set that has
    them all, so squash (Ln,Exp) and the rounds (Exp) never swap table
    sets. Set positions are preserved so emitted set ids stay valid."""
    import functools
    import concourse.hw_specs as hw
    import concourse.mybir as mybir
    if getattr(hw, "_capsule_act_patch", False):
        return
    A = mybir.ActivationFunctionType
    orig = hw.get_activation_tables

    @functools.cache
    def patched(arch):
        keep = {A.Exp, A.Ln, A.Square, A.Identity}
        out = {}
        for name, fns in orig(arch).items():
            if name == "natural_log_exp_and_others":
                out[name] = set(fns)
            else:
                out[name] = set(fns) - keep
        return out

    hw.get_activation_tables = patched
    # bacc binds the name at import time; patch that reference too
    import concourse.bacc as bacc
    bacc.get_activation_tables = patched
    hw._capsule_act_patch = True


def _build_bass():
    import concourse.bass as bass
    import concourse.bacc as bacc
    import concourse.mybir as mybir
    import concourse.tile as tile

    _patch_act_tables()

    f32 = mybir.dt.float32
    bf16 = mybir.dt.bfloat16
    nc = bacc.Bacc()

    wd = nc.dram_tensor("w", [T // WQ, 128, WQ * FREE], bf16, kind="ExternalInput")
    xtd = nc.dram_tensor("xt", [128, T, BL], bf16, kind="ExternalInput")
    xblkd = nc.dram_tensor("xblk", [128, T, 128], bf16, kind="ExternalInput")
    onesd = nc.dram_tensor("ones", [128, BL], bf16, kind="ExternalInput")
    onestd = nc.dram_tensor("onest", [BL, 128], bf16, kind="ExternalInput")
    outd = nc.dram_tensor("out", [BL, FREE], f32, kind="ExternalOutput")

    AX = mybir.AxisListType
    ALU = mybir.AluOpType
    ACTF = mybir.ActivationFunctionType

    with tile.TileContext(nc) as tc:
        with (
            tc.tile_pool(name="const", bufs=1) as constp,
            tc.tile_pool(name="u16", bufs=1) as up,
            tc.tile_pool(name="vexp", bufs=1) as vexpp,
            tc.tile_pool(name="psum_s", bufs=2, space="PSUM") as psum_s,
            tc.tile_pool(name="psum_v", bufs=1, space="PSUM") as psum_v,
        ):
            eps_sb = constp.tile([128, 1], f32)
            nc.gpsimd.memset(eps_sb[:], EPS)
            xt_sb = constp.tile([128, T, BL], bf16)
            ones_sb = constp.tile([128, BL], bf16)
            onest_sb = constp.tile([BL, 128], bf16)

            u16 = up.tile([128, T, FREE], bf16)

            # ---------------- pass 0: u_hat + s0 ----------------
            s0_ps = psum_s.tile([BL, FREE], f32, tag="s_ps")
            with (
                tc.tile_pool(name="xblk", bufs=1) as xblkp,
                tc.tile_pool(name="wt", bufs=4) as wtp,
                tc.tile_pool(name="psum_u", bufs=4, space="PSUM") as psum_u,
            ):
                # block-diag x built host-side: xblk[g*8+d, t, g*8+b] = x[b, t*16+g, d]
                # chunked so tile-0's stationary doesn't wait on a 4 MiB DMA
                XCH = T // 4
                xblk_c = []
                for q in range(4):
                    xc = xblkp.tile([128, XCH, 128], bf16, tag=f"xblk{q}")
                    # gpsimd (SWDGE) queue: runs parallel to the W DMAs on sync
                    nc.gpsimd.dma_start(xc[:], xblkd[:, q * XCH:(q + 1) * XCH, :])
                    xblk_c.append(xc)

                def xblk_ap(t):
                    return xblk_c[t // XCH][:, t % XCH, :]
                wt4 = None
                for t in range(T):
                    q, j = divmod(t, WQ)
                    if j == 0:
                        wt4 = wtp.tile([128, WQ, FREE], bf16, tag="wt4")
                        nc.sync.dma_start(wt4[:], wd[q])
                        if q == 0:
                            # consts ride the sync queue behind W chunk 0 so
                            # the first matmul isn't gated on them
                            nc.sync.dma_start(xt_sb[:], xtd[:])
                            nc.sync.dma_start(ones_sb[:], onesd[:])
                            nc.sync.dma_start(onest_sb[:], onestd[:])
                    ut_ps = psum_u.tile([128, FREE], f32, tag="u_ps")
                    nc.tensor.matmul(ut_ps[:], xblk_ap(t), wt4[:, j, :])
                    # s0 accumulation straight from x/32, W (fp32-exact in PSUM)
                    nc.tensor.matmul(
                        s0_ps[:], xt_sb[:, t, :], wt4[:, j, :],
                        start=(t == 0), stop=(t == T - 1),
                    )
                    # PSUM -> SBUF bf16 cast copy, weighted split ACT/DVE
                    if t % 7 < 4:
                        nc.scalar.copy(u16[:, t, :], ut_ps[:])
                    else:
                        nc.vector.tensor_copy(u16[:, t, :], ut_ps[:])

            # ---------------- squash (ln/exp; no sqrt table) ----------------
            with tc.tile_pool(name="sq", bufs=1) as sqp:

                def squash_and_bcast(s_ps, last, tag="v0"):
                    """v = squash(s_ps); write vexp [128, FREE] bf16, or DMA
                    fp32 v to outd if last. Returns vexp tile or None."""
                    sq2 = sqp.tile([BL, O, K], f32, tag="sq2")
                    nc.scalar.activation(
                        sq2[:], s_ps[:].rearrange("p (k o) -> p o k", o=O),
                        ACTF.Square)
                    s2 = sqp.tile([BL, O], f32, tag="s2")
                    nc.vector.reduce_sum(s2[:], sq2[:], axis=AX.X)
                    lnt = sqp.tile([BL, O], f32, tag="lnt")
                    nc.scalar.activation(lnt[:], s2[:], ACTF.Ln, bias=eps_sb[:BL])
                    rt = sqp.tile([BL, O], f32, tag="rt")
                    nc.scalar.activation(rt[:], lnt[:], ACTF.Exp, scale=0.5)
                    # den = (s2 + 1) * rt
                    den = sqp.tile([BL, O], f32, tag="den")
                    nc.vector.scalar_tensor_tensor(
                        den[:], s2[:], 1.0, rt[:], ALU.add, ALU.mult)
                    rden = sqp.tile([BL, O], f32, tag="rden")
                    nc.vector.reciprocal(rden[:], den[:])
                    scl = sqp.tile([BL, O], f32, tag="scl")
                    nc.vector.tensor_mul(scl[:], s2[:], rden[:])
                    # v = s * scl (broadcast over k)
                    v = sqp.tile([BL, K, O], f32 if last else bf16, tag="v")
                    nc.vector.tensor_mul(
                        v[:], s_ps[:].rearrange("p (k o) -> p k o", o=O),
                        scl[:].unsqueeze(1).broadcast_to([BL, K, O]))
                    if last:
                        nc.gpsimd.dma_start(outd[:], v[:].rearrange("p k o -> p (k o)"))
                        return None
                    # replicate v to all 16 partition groups via PE
                    vrep_ps = psum_v.tile([128, FREE], f32, tag="vrep")
                    nc.tensor.matmul(
                        vrep_ps[:], onest_sb[:],
                        v[:].rearrange("p k o -> p (k o)"))
                    vexp1 = vexpp.tile([128, FREE], bf16, tag=tag)
                    nc.scalar.copy(vexp1[:], vrep_ps[:])
                    return vexp1

                vexp0 = squash_and_bcast(s0_ps, last=False, tag="v0")

                # ---------------- rounds 1, 2 (pipelined) ----------------
                with (
                    tc.tile_pool(name="scr", bufs=1) as scr,
                    tc.tile_pool(name="epool", bufs=2) as ep,
                    tc.tile_pool(name="cupool", bufs=2) as cup,
                ):
                    for rnd in (1, 2):
                        if rnd == 1:
                            vin = vexp0
                        else:
                            # b2 = u.(v0+v1): one vu pass against v0+v1
                            vin = vexpp.tile([128, FREE], bf16, tag="vsum")
                            nc.vector.tensor_add(vin[:], vexp0[:], vexp1[:])
                        s_ps = psum_s.tile([BL, FREE], f32, tag="s_ps")
                        vview = vin[:].rearrange(
                            "p (k o) -> p k o", o=O).unsqueeze(1).broadcast_to(
                            [128, B8, K, O])
                        pending = None

                        CH = B8 // 2

                        def tail(p):
                            tb, e_t = p
                            z = scr.tile([128, B8], f32, tag="z")
                            nc.vector.reduce_sum(z[:], e_t[:], axis=AX.X)
                            rz = scr.tile([128, B8], f32, tag="rz")
                            nc.vector.reciprocal(rz[:], z[:])
                            c = scr.tile([128, B8, O], bf16, tag="c")
                            nc.vector.tensor_mul(
                                c[:], e_t[:],
                                rz[:].unsqueeze(-1).broadcast_to([128, B8, O]))
                            # two half-chunks through a 2-deep rotating pool:
                            # matmuls of chunk N never block chunk N+1's write
                            for ch in range(2):
                                j0 = ch * CH
                                cu = cup.tile([128, CH, K, O], bf16, tag="cu")
                                nc.vector.tensor_mul(
                                    cu[:],
                                    u16[:, tb + j0:tb + j0 + CH, :].rearrange(
                                        "p t (k o) -> p t k o", o=O),
                                    c[:, j0:j0 + CH].unsqueeze(2).broadcast_to(
                                        [128, CH, K, O]))
                                for jj in range(CH):
                                    t = tb + j0 + jj
                                    nc.tensor.matmul(
                                        s_ps[:], ones_sb[:],
                                        cu[:, jj, :, :].rearrange(
                                            "p k o -> p (k o)"),
                                        start=(t == 0), stop=(t == T - 1))

                        for ti in range(NB):
                            tb = ti * B8
                            u_sl = u16[:, tb:tb + B8, :].rearrange(
                                "p t (k o) -> p t k o", o=O)
                            vu = scr.tile([128, B8, K, O], bf16, tag="vu")
                            nc.vector.tensor_mul(vu[:], u_sl, vview)
                            t1 = scr.tile([128, B8, 8, O], bf16, tag="t1")
                            nc.vector.tensor_add(
                                t1[:], vu[:, :, 0:8], vu[:, :, 8:16])
                            t2 = scr.tile([128, B8, 4, O], bf16, tag="t2")
                            nc.vector.tensor_add(
                                t2[:], t1[:, :, 0:4], t1[:, :, 4:8])
                            t3 = scr.tile([128, B8, 2, O], bf16, tag="t3")
                            nc.vector.tensor_add(
                                t3[:], t2[:, :, 0:2], t2[:, :, 2:4])
                            lg = scr.tile([128, B8, O], bf16, tag="lg")
                            nc.vector.tensor_add(
                                lg[:], t3[:, :, 0, :], t3[:, :, 1, :])
                            e_t = ep.tile([128, B8, O], bf16, tag="e")
                            nc.scalar.activation(e_t[:], lg[:], ACTF.Exp)
                            if pending is not None:
                                tail(pending)
                            pending = (tb, e_t)
                        tail(pending)
                        vexp1 = squash_and_bcast(
                            s_ps, last=(rnd == 2), tag="v1")
    nc.finalize()
    return nc


def _host_prep():
    """Core-independent input prep pieces."""
    ones = np.zeros((128, BL), dtype=BF16)
    for g in range(G):
        for b in range(BL):
            ones[g * 8 + b, b] = 1
    onest = np.ascontiguousarray(ones.T)
    return ones, onest


def kernel(x: np.ndarray, W: np.ndarray) -> np.ndarray:
    from concourse import bass_utils

    if "nc" not in _CACHE:
        _CACHE["nc"] = _build_bass()
        _CACHE["ones"], _CACHE["onest"] = _host_prep()
    nc = _CACHE["nc"]

    # W -> [T, (g,d), (k,o)] : w[t, g*8+d, k*32+o] = W[t*16+g, o, d, k]
    # then group 4 tiles per DMA: [T//4, 128, 4*FREE]
    wr = (W.reshape(T, G, O, D, K).transpose(0, 1, 3, 4, 2)
          .reshape(T, 128, FREE))
    wr4 = np.ascontiguousarray(
        wr.reshape(T // WQ, WQ, 128, FREE).transpose(0, 2, 1, 3)
        .reshape(T // WQ, 128, WQ * FREE)).astype(BF16)
    in_maps = []
    for c in range(NC_N):
        xl = x[c * BL:(c + 1) * BL]  # [8, 2048, 8]
        # xt[g*8+d, t, b] = xl[b, t*16+g, d] / 32  (folds round-0 c=1/O)
        xt = np.ascontiguousarray(
            xl.reshape(BL, T, G, D).transpose(2, 3, 1, 0).reshape(128, T, BL)
        ) * (1.0 / O)
        xt = xt.astype(BF16)
        xtu = np.ascontiguousarray(
            xl.reshape(BL, T, G, D).transpose(2, 3, 1, 0).reshape(128, T, BL)
        ).astype(BF16)
        xblk = np.zeros((128, T, 128), dtype=BF16)
        for g in range(G):
            xblk[g * 8:(g + 1) * 8, :, g * 8:(g + 1) * 8] = xtu[g * 8:(g + 1) * 8]
        in_maps.append({"w": wr4, "xt": xt, "xblk": xblk, "ones": _CACHE["ones"],
                        "onest": _CACHE["onest"]})

    _CACHE["in_maps"] = in_maps
    res = bass_utils.run_bass_kernel_spmd(nc, in_maps, core_ids=list(range(NC_N)))
    out = np.empty((B, O, K), np.float32)
    for c in range(NC_N):
        v = res.results[c]["out"].reshape(BL, K, O)  # (k,o) cols
        out[c * BL:(c + 1) * BL] = v.transpose(0, 2, 1)
    return out


# revision 30
# speedup vs baseline: 1.0104x; 1.0043x over previous
"""CapsuleLayer (dynamic routing, 3 iterations) Trainium2 Bass kernel.

Problem (hardcoded):
    x: [64, 2048, 8] f32, W: [2048, 32, 8, 16] f32
    u_hat[b,o,i,k] = sum_d x[b,i,d] * W[i,o,d,k]
    3 rounds of routing-by-agreement over logits b[B,O,I], softmax over O.
    out v: [64, 32, 16] f32.

Sharding: data-parallel over batch across 8 cores (8 batch rows each), W
replicated. Per-core layout: partitions = (g,b) with g=16 i's per tile,
free = (k,o); T=128 tiles of 16 i's.

Pass 0: per tile, PE matmul with block-diag x produces u_hat tile; a second
accumulating matmul (lhsT = x/32) produces s0 directly. PSUM->SBUF bf16
copies split between ACT and DVE. W DMA'd on the Sync queue 4 tiles/DMA.

Rounds 1,2 (software-pipelined, batch of 8 tiles, skew-1 tail):
  V: vu = u*v (bcast), k-tree reduce -> logits; S: batched exp (no accum);
  V-tail: z = reduce_sum(e), rz = 1/z, c = e*rz (bcast), cu = u*c;
  T-tail: 8 ones-matmuls accumulate s in PSUM.
Squash uses ln/exp instead of sqrt (one ACT table set for the whole kernel).
"""

import numpy as np
import ml_dtypes

BF16 = ml_dtypes.bfloat16

B, I, D, O, K = 64, 2048, 8, 32, 16
NC_N = 8              # cores
BL = B // NC_N        # 8 batch rows per core
G = 16                # i's per tile
T = I // G            # 128 tiles
FREE = O * K          # 512, layout (k,o): col = k*32+o
EPS = 1e-7
B8 = 16               # tiles per round-loop iteration
NB = T // B8          # iterations per round
WQ = 8                # W tiles per DMA

_CACHE = {}


def _patch_act_tables():
    """Constrain Exp/Ln/Square/Identity to the one ACT table set that has
    them all, so squash (Ln,Exp) and the rounds (Exp) never swap table
    sets. Set positions are preserved so emitted set ids stay valid."""
    import functools
    import concourse.hw_specs as hw
    import concourse.mybir as mybir
    if getattr(hw, "_capsule_act_patch", False):
        return
    A = mybir.ActivationFunctionType
    orig = hw.get_activation_tables

    @functools.cache
    def patched(arch):
        keep = {A.Exp, A.Ln, A.Square, A.Identity}
        out = {}
        for name, fns in orig(arch).items():
            if name == "natural_log_exp_and_others":
                out[name] = set(fns)
            else:
                out[name] = set(fns) - keep
        return out

    hw.get_activation_tables = patched
    # bacc binds the name at import time; patch that reference too
    import concourse.bacc as bacc
    bacc.get_activation_tables = patched
    hw._capsule_act_patch = True


def _build_bass():
    import concourse.bass as bass
    import concourse.bacc as bacc
    import concourse.mybir as mybir
    import concourse.tile as tile

    _patch_act_tables()

    f32 = mybir.dt.float32
    bf16 = mybir.dt.bfloat16
    nc = bacc.Bacc()

    wd = nc.dram_tensor("w", [T // WQ, 128, WQ * FREE], bf16, kind="ExternalInput")
    xtd = nc.dram_tensor("xt", [128, T, BL], bf16, kind="ExternalInput")
    xblkd = nc.dram_tensor("xblk", [128, T, 128], bf16, kind="ExternalInput")
    onesd = nc.dram_tensor("ones", [128, BL], bf16, kind="ExternalInput")
    onestd = nc.dram_tensor("onest", [BL, 128], bf16, kind="ExternalInput")
    outd = nc.dram_tensor("out", [BL, FREE], f32, kind="ExternalOutput")

    AX = mybir.AxisListType
    ALU = mybir.AluOpType
    ACTF = mybir.ActivationFunctionType

    with tile.TileContext(nc) as tc:
        with (
            tc.tile_pool(name="const", bufs=1) as constp,
            tc.tile_pool(name="u16", bufs=1) as up,
            tc.tile_pool(name="vexp", bufs=1) as vexpp,
            tc.tile_pool(name="psum_s", bufs=2, space="PSUM") as psum_s,
            tc.tile_pool(name="psum_v", bufs=1, space="PSUM") as psum_v,
        ):
            eps_sb = constp.tile([128, 1], f32)
            nc.gpsimd.memset(eps_sb[:], EPS)
            xt_sb = constp.tile([128, T, BL], bf16)
            ones_sb = constp.tile([128, BL], bf16)
            onest_sb = constp.tile([BL, 128], bf16)

            u16 = up.tile([128, T, FREE], bf16)

            # ---------------- pass 0: u_hat + s0 ----------------
            s0_ps = psum_s.tile([BL, FREE], f32, tag="s_ps")
            with (
                tc.tile_pool(name="xblk", bufs=1) as xblkp,
                tc.tile_pool(name="wt", bufs=3) as wtp,
                tc.tile_pool(name="psum_u", bufs=4, space="PSUM") as psum_u,
            ):
                # block-diag x built host-side: xblk[g*8+d, t, g*8+b] = x[b, t*16+g, d]
                # chunked so tile-0's stationary doesn't wait on a 4 MiB DMA
                XCH = T // 4
                xblk_c = []
                for q in range(4):
                    xc = xblkp.tile([128, XCH, 128], bf16, tag=f"xblk{q}")
                    # gpsimd (SWDGE) queue: runs parallel to the W DMAs on sync
                    nc.gpsimd.dma_start(xc[:], xblkd[:, q * XCH:(q + 1) * XCH, :])
                    xblk_c.append(xc)

                def xblk_ap(t):
                    return xblk_c[t // XCH][:, t % XCH, :]
                wt4 = None
                for t in range(T):
                    q, j = divmod(t, WQ)
                    if j == 0:
                        wt4 = wtp.tile([128, WQ, FREE], bf16, tag="wt4")
                        nc.sync.dma_start(wt4[:], wd[q])
                        if q == 0:
                            # consts ride the sync queue behind W chunk 0 so
                            # the first matmul isn't gated on them
                            nc.sync.dma_start(xt_sb[:], xtd[:])
                            nc.sync.dma_start(ones_sb[:], onesd[:])
                            nc.sync.dma_start(onest_sb[:], onestd[:])
                    ut_ps = psum_u.tile([128, FREE], f32, tag="u_ps")
                    nc.tensor.matmul(ut_ps[:], xblk_ap(t), wt4[:, j, :])
                    # s0 accumulation straight from x/32, W (fp32-exact in PSUM)
                    nc.tensor.matmul(
                        s0_ps[:], xt_sb[:, t, :], wt4[:, j, :],
                        start=(t == 0), stop=(t == T - 1),
                    )
                    # PSUM -> SBUF bf16 cast copy, weighted split ACT/DVE
                    if t % 7 < 4:
                        nc.scalar.copy(u16[:, t, :], ut_ps[:])
                    else:
                        nc.vector.tensor_copy(u16[:, t, :], ut_ps[:])

            # ---------------- squash (ln/exp; no sqrt table) ----------------
            with tc.tile_pool(name="sq", bufs=1) as sqp:

                def squash_and_bcast(s_ps, last, tag="v0"):
                    """v = squash(s_ps); write vexp [128, FREE] bf16, or DMA
                    fp32 v to outd if last. Returns vexp tile or None."""
                    sq2 = sqp.tile([BL, O, K], f32, tag="sq2")
                    nc.scalar.activation(
                        sq2[:], s_ps[:].rearrange("p (k o) -> p o k", o=O),
                        ACTF.Square)
                    s2 = sqp.tile([BL, O], f32, tag="s2")
                    nc.vector.reduce_sum(s2[:], sq2[:], axis=AX.X)
                    lnt = sqp.tile([BL, O], f32, tag="lnt")
                    nc.scalar.activation(lnt[:], s2[:], ACTF.Ln, bias=eps_sb[:BL])
                    rt = sqp.tile([BL, O], f32, tag="rt")
                    nc.scalar.activation(rt[:], lnt[:], ACTF.Exp, scale=0.5)
                    # den = (s2 + 1) * rt
                    den = sqp.tile([BL, O], f32, tag="den")
                    nc.vector.scalar_tensor_tensor(
                        den[:], s2[:], 1.0, rt[:], ALU.add, ALU.mult)
                    rden = sqp.tile([BL, O], f32, tag="rden")
                    nc.vector.reciprocal(rden[:], den[:])
                    scl = sqp.tile([BL, O], f32, tag="scl")
                    nc.vector.tensor_mul(scl[:], s2[:], rden[:])
                    # v = s * scl (broadcast over k)
                    v = sqp.tile([BL, K, O], f32 if last else bf16, tag="v")
                    nc.vector.tensor_mul(
                        v[:], s_ps[:].rearrange("p (k o) -> p k o", o=O),
                        scl[:].unsqueeze(1).broadcast_to([BL, K, O]))
                    if last:
                        nc.gpsimd.dma_start(outd[:], v[:].rearrange("p k o -> p (k o)"))
                        return None
                    # replicate v to all 16 partition groups via PE
                    vrep_ps = psum_v.tile([128, FREE], f32, tag="vrep")
                    nc.tensor.matmul(
                        vrep_ps[:], onest_sb[:],
                        v[:].rearrange("p k o -> p (k o)"))
                    vexp1 = vexpp.tile([128, FREE], bf16, tag=tag)
                    nc.scalar.copy(vexp1[:], vrep_ps[:])
                    return vexp1

                vexp0 = squash_and_bcast(s0_ps, last=False, tag="v0")

                # ---------------- rounds 1, 2 (pipelined) ----------------
                with (
                    tc.tile_pool(name="scr", bufs=1) as scr,
                    tc.tile_pool(name="epool", bufs=2) as ep,
                    tc.tile_pool(name="cupool", bufs=2) as cup,
                ):
                    for rnd in (1, 2):
                        if rnd == 1:
                            vin = vexp0
                        else:
                            # b2 = u.(v0+v1): one vu pass against v0+v1
                            vin = vexpp.tile([128, FREE], bf16, tag="vsum")
                            nc.vector.tensor_add(vin[:], vexp0[:], vexp1[:])
                        s_ps = psum_s.tile([BL, FREE], f32, tag="s_ps")
                        vview = vin[:].rearrange(
                            "p (k o) -> p k o", o=O).unsqueeze(1).broadcast_to(
                            [128, B8, K, O])
                        pending = None

                        CH = B8 // 2

                        def tail(p):
                            tb, e_t = p
                            z = scr.tile([128, B8], f32, tag="z")
                            nc.vector.reduce_sum(z[:], e_t[:], axis=AX.X)
                            rz = scr.tile([128, B8], f32, tag="rz")
                            nc.vector.reciprocal(rz[:], z[:])
                            c = scr.tile([128, B8, O], bf16, tag="c")
                            nc.vector.tensor_mul(
                                c[:], e_t[:],
                                rz[:].unsqueeze(-1).broadcast_to([128, B8, O]))
                            # two half-chunks through a 2-deep rotating pool:
                            # matmuls of chunk N never block chunk N+1's write
                            for ch in range(2):
                                j0 = ch * CH
                                cu = cup.tile([128, CH, K, O], bf16, tag="cu")
                                nc.vector.tensor_mul(
                                    cu[:],
                                    u16[:, tb + j0:tb + j0 + CH, :].rearrange(
                                        "p t (k o) -> p t k o", o=O),
                                    c[:, j0:j0 + CH].unsqueeze(2).broadcast_to(
                                        [128, CH, K, O]))
                                for jj in range(CH):
                                    t = tb + j0 + jj
                                    nc.tensor.matmul(
                                        s_ps[:], ones_sb[:],
                                        cu[:, jj, :, :].rearrange(
                                            "p k o -> p (k o)"),
                                        start=(t == 0), stop=(t == T - 1))

                        for ti in range(NB):
                            tb = ti * B8
                            u_sl = u16[:, tb:tb + B8, :].rearrange(
                                "p t (k o) -> p t k o", o=O)
                            vu = scr.tile([128, B8, K, O], bf16, tag="vu")
                            nc.vector.tensor_mul(vu[:], u_sl, vview)
                            t1 = scr.tile([128, B8, 8, O], bf16, tag="t1")
                            nc.vector.tensor_add(
                                t1[:], vu[:, :, 0:8], vu[:, :, 8:16])
                            t2 = scr.tile([128, B8, 4, O], bf16, tag="t2")
                            nc.vector.tensor_add(
                                t2[:], t1[:, :, 0:4], t1[:, :, 4:8])
                            t3 = scr.tile([128, B8, 2, O], bf16, tag="t3")
                            nc.vector.tensor_add(
                                t3[:], t2[:, :, 0:2], t2[:, :, 2:4])
                            lg = scr.tile([128, B8, O], bf16, tag="lg")
                            nc.vector.tensor_add(
                                lg[:], t3[:, :, 0, :], t3[:, :, 1, :])
                            e_t = ep.tile([128, B8, O], bf16, tag="e")
                            nc.scalar.activation(e_t[:], lg[:], ACTF.Exp)
                            if pending is not None:
                                tail(pending)
                            pending = (tb, e_t)
                        tail(pending)
                        vexp1 = squash_and_bcast(
                            s_ps, last=(rnd == 2), tag="v1")
    nc.finalize()
    return nc


def _host_prep():
    """Core-independent input prep pieces."""
    ones = np.zeros((128, BL), dtype=BF16)
    for g in range(G):
        for b in range(BL):
            ones[g * 8 + b, b] = 1
    onest = np.ascontiguousarray(ones.T)
    return ones, onest


def kernel(x: np.ndarray, W: np.ndarray) -> np.ndarray:
    from concourse import bass_utils

    if "nc" not in _CACHE:
        _CACHE["nc"] = _build_bass()
        _CACHE["ones"], _CACHE["onest"] = _host_prep()
    nc = _CACHE["nc"]

    # W -> [T, (g,d), (k,o)] : w[t, g*8+d, k*32+o] = W[t*16+g, o, d, k]
    # then group 4 tiles per DMA: [T//4, 128, 4*FREE]
    wr = (W.reshape(T, G, O, D, K).transpose(0, 1, 3, 4, 2)
          .reshape(T, 128, FREE))
    wr4 = np.ascontiguousarray(
        wr.reshape(T // WQ, WQ, 128, FREE).transpose(0, 2, 1, 3)
        .reshape(T // WQ, 128, WQ * FREE)).astype(BF16)
    in_maps = []
    for c in range(NC_N):
        xl = x[c * BL:(c + 1) * BL]  # [8, 2048, 8]
        # xt[g*8+d, t, b] = xl[b, t*16+g, d] / 32  (folds round-0 c=1/O)
        xt = np.ascontiguousarray(
            xl.reshape(BL, T, G, D).transpose(2, 3, 1, 0).reshape(128, T, BL)
        ) * (1.0 / O)
        xt = xt.astype(BF16)
        xtu = np.ascontiguousarray(
            xl.reshape(BL, T, G, D).transpose(2, 3, 1, 0).reshape(128, T, BL)
        ).astype(BF16)
        xblk = np.zeros((128, T, 128), dtype=BF16)
        for g in range(G):
            xblk[g * 8:(g + 1) * 8, :, g * 8:(g + 1) * 8] = xtu[g * 8:(g + 1) * 8]
        in_maps.append({"w": wr4, "xt": xt, "xblk": xblk, "ones": _CACHE["ones"],
                        "onest": _CACHE["onest"]})

    _CACHE["in_maps"] = in_maps
    res = bass_utils.run_bass_kernel_spmd(nc, in_maps, core_ids=list(range(NC_N)))
    out = np.empty((B, O, K), np.float32)
    for c in range(NC_N):
        v = res.results[c]["out"].reshape(BL, K, O)  # (k,o) cols
        out[c * BL:(c + 1) * BL] = v.transpose(0, 2, 1)
    return out
